# revision 29
# baseline (speedup 1.0000x reference)
"""Trainium2 Bass kernel for the scatter_memory problem.

Full (unsharded) inputs in, full output out. Internally: 8-way shard over
(batch, window-half); pair-wise AllReduce combines softmax partials.

Math restructuring vs the reference (validated to rel err 5e-6 in fp32):
  - the self-attention branch (sa_*) is dead code -> skipped
  - summary feeds only the cross-attention; scores fold qa_q/sqrt(d) @ qa_wk
    into one [1024, 64] matrix on the host
  - softmax without max-subtraction (scores are in [-6, 6]); partial
    numerator/denominator sums are combined with a pair AllReduce
"""

import numpy as np

import concourse.bacc as bacc
import concourse.mybir as mybir
import concourse.tile as tile
import concourse.bass_utils as bass_utils

N_CORES = 8
DIM = 1024
L = 16            # SUMMARY_LEN
STRIDE = 8
NWIN = 512        # windows per batch
NLOC = 256        # windows per core (half a batch)
XLOC = NLOC * STRIDE + (L - STRIDE)   # 2056 x-positions per core
CONV = 4104       # padded seq len
EPS = 1.1920929e-07
BF16 = True       # dtype of the big-projection operands (x windows, ws_w)

_DT = mybir.dt.bfloat16 if BF16 else mybir.dt.float32
_NPDT = np.dtype("bfloat16") if BF16 else np.dtype("float32")


def build_nc(reps: int = 1, use_collective: bool = True, phase: str = "full", safe=()):
    """Build the per-core Bass module. `reps` statically repeats the whole
    body (for wall-clock-delta timing). With use_collective=False the pair
    combine becomes a local copy (for single-core simulation). `phase` can
    truncate the kernel after "proj" or "attn" for profiling experiments."""
    f32 = mybir.dt.float32
    f32r = mybir.dt.float32r
    nc = bacc.Bacc("TRN2", target_bir_lowering=False, debug=False,
                   num_devices=N_CORES)

    xt_d = nc.dram_tensor("xt", [8, 128, XLOC], _DT, kind="ExternalInput")
    wt_d = nc.dram_tensor("wt", [32, 128, 4096], _DT, kind="ExternalInput")
    wv_d = nc.dram_tensor("wv", [8, 128, 1024], _DT, kind="ExternalInput")
    cq_d = nc.dram_tensor("cq", [8, 128, 64], _DT, kind="ExternalInput")
    h_d = nc.dram_tensor("hb", [64, 1024], f32, kind="ExternalInput")
    mn_d = nc.dram_tensor("mnw", [64, 1024], f32, kind="ExternalInput")
    hn_d = nc.dram_tensor("hnw", [64, 1024], f32, kind="ExternalInput")
    id_d = nc.dram_tensor("ident", [64, 64], f32, kind="ExternalInput")
    out_d = nc.dram_tensor("out", [64, 1024], f32, kind="ExternalOutput")

    with tile.TileContext(nc) as tc:
        with (
            tc.tile_pool(name="const", bufs=1) as cpool,
            tc.tile_pool(name="x", bufs=1) as xpool,
            tc.tile_pool(name="w", bufs=6) as wpool,
            tc.tile_pool(name="sm", bufs=1) as spool,
            tc.tile_pool(name="small", bufs=1) as mpool,
            tc.tile_pool(name="ps", bufs=8, space="PSUM") as ppool,
            tc.tile_pool(name="dram", bufs=2, space="DRAM") as dpool,
        ):
            ident = cpool.tile([64, 64], f32, tag="ident")
            nc.sync.dma_start(ident[:], id_d[:])
            ones = cpool.tile([128, 64], f32, tag="ones")
            nc.vector.memset(ones[:], 1.0)
            eps_sb = cpool.tile([1, 1], f32, tag="eps")
            nc.vector.memset(eps_sb[:], EPS)
            # prime the ACT function tables (Sqrt/Exp/Square) off the
            # critical path so no LoadActFuncSet lands mid-epilogue
            warm = cpool.tile([1, 1], f32, tag="warm")
            if "warm" not in safe:
                nc.scalar.activation(warm[:], eps_sb[:],
                                     mybir.ActivationFunctionType.Sqrt)
                nc.scalar.activation(warm[:], eps_sb[:],
                                     mybir.ActivationFunctionType.Exp)
                nc.scalar.activation(warm[:], eps_sb[:],
                                     mybir.ActivationFunctionType.Square)

            for _rep in range(reps):
                # ---- x chunks as 8 separate tiles; DMA issue order is
                # interleaved with the first W chunks so the projection's
                # first matmuls start as early as possible (the serial DMA
                # pipe processes in issue order). Even g uses xts[0..3],
                # odd g uses xts[4..7]. ----
                xts = [xpool.tile([128, XLOC], _DT, tag=f"xts{dc}",
                                  name=f"xts{dc}") for dc in range(8)]
                wchs = {}
                wchs[0] = wpool.tile([128, 4096], _DT, tag="wch", name="wch0")
                nc.sync.dma_start(wchs[0][:], wt_d[0])
                nc.sync.dma_start(xts[0][:], xt_d[0])

                # ---- projection: summaryT[m, n] = sum_f wsT[f, m] win[n, f] ----
                # remaining inputs are DMA'd mid-loop so they overlap the
                # projection instead of delaying its first matmul
                ps = [ppool.tile([128, 256], f32, tag="ps", name=f"ps{i}") for i in range(8)]
                wv_sb = cpool.tile([128, 8, 1024], _DT, tag="wv")
                cq_sb = cpool.tile([128, 8, 64], _DT, tag="cq")
                for g in range(32):
                    if g in wchs:
                        wch = wchs[g]
                    else:
                        wch = wpool.tile([128, 4096], _DT, tag="wch")
                        nc.sync.dma_start(wch[:], wt_d[g])
                    for j in range(4):
                        k = g * 4 + j
                        dc, l = k // 16, k % 16   # dc-major order: chunk g
                        # needs only xts[g//4] (+1 chunk), not all 8
                        rhs = xts[dc][:, l:l + 2041:8]   # [128, 256] stride-8 view
                        for mt in range(8):
                            nc.tensor.matmul(
                                ps[mt][:], wch[:, j * 1024 + mt * 128:j * 1024 + (mt + 1) * 128],
                                rhs, start=(k == 0), stop=(k == 127))
                    # just-in-time loads between W chunks: the serial DMA
                    # pipe has ~0.5us/chunk headroom over the PE's consumption
                    # rate, so each extra load is placed where that headroom
                    # has accumulated. xts[dc] is first used at chunk 4*dc.
                    if g % 4 == 0 and g // 4 + 1 <= 7:
                        dc_next = g // 4 + 1
                        nc.sync.dma_start(xts[dc_next][:], xt_d[dc_next])
                    if 14 <= g <= 28 and g % 2 == 0:
                        mt = (g - 14) // 2
                        nc.sync.dma_start(wv_sb[:, mt, :], wv_d[mt])
                    elif g == 29:
                        nc.sync.dma_start(cq_sb[:], cq_d[:].rearrange("a b c -> b a c"))
                        h_sb = mpool.tile([64, 1024], f32, tag="h")
                        nc.sync.dma_start(h_sb[:], h_d[:])
                    elif g == 30:
                        mn_sb = mpool.tile([64, 1024], f32, tag="mn")
                        nc.sync.dma_start(mn_sb[:], mn_d[:])
                        hn_sb = mpool.tile([64, 1024], f32, tag="hn")
                        nc.sync.dma_start(hn_sb[:], hn_d[:])
                sm = [spool.tile([128, 256], _DT, tag=f"sm{mt}", name=f"sm{mt}") for mt in range(8)]
                for mt in range(8):
                    nc.vector.tensor_copy(sm[mt][:], ps[mt][:])
                if phase == "proj":
                    nc.sync.dma_start(out_d[0:64, 0:128].bitcast(mybir.dt.bfloat16), sm[0][0:64, :])
                    continue

                # ---- scores[q, n] (pre-scaled), exp, row sums ----
                sc_ps = ppool.tile([64, 256], f32, tag="ps")
                for mt in range(8):
                    nc.tensor.matmul(sc_ps[:], cq_sb[:, mt, :], sm[mt][:],
                                     start=(mt == 0), stop=(mt == 7))
                p_sb = mpool.tile([64, 256], f32, tag="p")
                sloc = mpool.tile([64, 1], f32, tag="sloc")
                nc.scalar.activation(p_sb[:], sc_ps[:],
                                     mybir.ActivationFunctionType.Exp,
                                     accum_out=sloc[:])
                if "warm" not in safe:
                    # hoist the Sqrt act-table switch off the epilogue chain
                    nc.scalar.activation(warm[:], eps_sb[:],
                                         mybir.ActivationFunctionType.Sqrt)

                # ---- P^T via PE transpose ----
                pt_sb = []
                for nt in range(2):
                    tp = ppool.tile([128, 64], f32, tag="ps")
                    nc.tensor.transpose(tp[:], p_sb[:, nt * 128:(nt + 1) * 128],
                                        ident[:])
                    t_sb = mpool.tile([128, 64], f32r, tag=f"pt{nt}")
                    nc.vector.tensor_copy(t_sb[:], tp[:])
                    pt_sb.append(t_sb)

                # ---- qv[n, h] = summary @ qa_wv^T ----
                qv_sb = [spool.tile([128, 1024], f32r, tag=f"qv{nt}", name=f"qv{nt}") for nt in range(2)]
                for nt in range(2):
                    for hh in range(2):
                        qp = ppool.tile([128, 512], f32, tag="ps")
                        for mt in range(8):
                            nc.tensor.matmul(
                                qp[:], sm[mt][:, nt * 128:(nt + 1) * 128],
                                wv_sb[:, mt, hh * 512:(hh + 1) * 512],
                                start=(mt == 0), stop=(mt == 7))
                        nc.vector.tensor_copy(qv_sb[nt][:, hh * 512:(hh + 1) * 512], qp[:])

                # ---- out_loc[q, h] = P^T.T @ qv (unnormalized); DMA the
                # PSUM halves + denominator straight into the collective's
                # DRAM input ----
                cin = dpool.tile([64, 1025], f32, tag="cin")
                payload = mpool.tile([64, 1025], f32, tag="payload")
                for hh in range(2):
                    op = ppool.tile([64, 512], f32, tag="ps")
                    for nt in range(2):
                        nc.tensor.matmul(op[:], pt_sb[nt][:],
                                         qv_sb[nt][:, hh * 512:(hh + 1) * 512],
                                         start=(nt == 0), stop=(nt == 1))
                    nc.vector.tensor_copy(payload[:, hh * 512:(hh + 1) * 512], op[:])
                nc.vector.tensor_copy(payload[:, 1024:1025], sloc[:])
                nc.sync.dma_start(cin[:], payload[:])
                if phase == "attn":
                    nc.sync.dma_start(out_d[:], cin[:, 0:1024])
                    continue

                # ---- pair AllReduce of (numerator, denominator) ----
                comb = mpool.tile([64, 1025], f32, tag="comb")
                if use_collective:
                    cout = dpool.tile([64, 1025], f32, tag="cout")
                    nc.gpsimd.collective_compute(
                        "AllReduce", mybir.AluOpType.add,
                        replica_groups=[[0, 1], [2, 3], [4, 5], [6, 7]],
                        ins=[cin.opt()], outs=[cout.opt()])
                    nc.sync.dma_start(comb[:], cout[:])
                else:
                    nc.sync.dma_start(comb[:], cin[:])

                # ---- memory = num / den; two full-tensor RMSNorms.
                # sum(mem^2) is computed from the *unnormalized* numerator
                # (sum(num^2) * rec^2) so the ACT square pass overlaps the
                # DVE normalize, and w-multiplies run while the scalar
                # rsqrt chain resolves. ----
                rec = mpool.tile([64, 1], f32, tag="rec")
                nc.vector.reciprocal(rec[:], comb[:, 1024:1025])
                scr0 = mpool.tile([64, 1024], f32, tag="scr0")
                sq0 = mpool.tile([64, 1], f32, tag="sq0")
                # NOTE: nc.vector.tensor_tensor_reduce hangs on HW (mesh
                # desync) under this toolchain -- use ACT Square+accum.
                nc.scalar.activation(scr0[:], comb[:, 0:1024],
                                     mybir.ActivationFunctionType.Square,
                                     accum_out=sq0[:])
                tmn = mpool.tile([64, 1024], f32, tag="tmn")
                if "stt" in safe:
                    nc.vector.tensor_scalar_mul(tmn[:], comb[:, 0:1024], rec[:])
                    nc.vector.tensor_mul(tmn[:], tmn[:], mn_sb[:])
                else:
                    nc.vector.scalar_tensor_tensor(
                        tmn[:], comb[:, 0:1024], rec[:], mn_sb[:],
                        op0=mybir.AluOpType.mult, op1=mybir.AluOpType.mult)
                sq0n = mpool.tile([64, 1], f32, tag="sq0n")
                if "ts2" in safe:
                    nc.vector.tensor_scalar_mul(sq0n[:], sq0[:], rec[:])
                    nc.vector.tensor_scalar_mul(sq0n[:], sq0n[:], rec[:])
                else:
                    nc.vector.tensor_scalar(sq0n[:], sq0[:], rec[:], rec[:],
                                            op0=mybir.AluOpType.mult,
                                            op1=mybir.AluOpType.mult)

                def rsqrt_mean(sq, idx):
                    # [64,1] per-row sums -> 1/sqrt(mean + EPS), broadcast [64,1]
                    msp = ppool.tile([1, 1], f32, tag="ps")
                    nc.tensor.matmul(msp[:], sq[:], ones[0:64, 0:1],
                                     start=True, stop=True)
                    std = mpool.tile([1, 1], f32, tag=f"std{idx}", name=f"std{idx}")
                    nc.scalar.activation(std[:], msp[:],
                                         mybir.ActivationFunctionType.Sqrt,
                                         scale=1.0 / 65536.0, bias=eps_sb[:])
                    bst = ppool.tile([64, 1], f32, tag="ps")
                    nc.tensor.matmul(bst[:], ones[0:1, 0:64], std[:],
                                     start=True, stop=True)
                    rstd = mpool.tile([64, 1], f32, tag=f"rstd{idx}", name=f"rstd{idx}")
                    nc.vector.reciprocal(rstd[:], bst[:])
                    return rstd

                rstd1 = rsqrt_mean(sq0n, 0)
                hh1 = mpool.tile([64, 1024], f32, tag="hh1")
                if "stt" in safe:
                    nc.vector.tensor_scalar_mul(hh1[:], tmn[:], rstd1[:])
                    nc.vector.tensor_add(hh1[:], hh1[:], h_sb[:])
                else:
                    nc.vector.scalar_tensor_tensor(
                        hh1[:], tmn[:], rstd1[:], h_sb[:],
                        op0=mybir.AluOpType.mult, op1=mybir.AluOpType.add)

                scr1 = mpool.tile([64, 1024], f32, tag="scr1")
                sq1 = mpool.tile([64, 1], f32, tag="sq1")
                nc.scalar.activation(scr1[:], hh1[:],
                                     mybir.ActivationFunctionType.Square,
                                     accum_out=sq1[:])
                thn = mpool.tile([64, 1024], f32, tag="thn")
                nc.vector.tensor_mul(thn[:], hh1[:], hn_sb[:])
                rstd2 = rsqrt_mean(sq1, 1)
                o = mpool.tile([64, 1024], f32, tag="o")
                nc.vector.tensor_scalar_mul(o[:], thn[:], rstd2[:])
                nc.sync.dma_start(out_d[:], o[:])

    nc.compile()
    return nc


def prep_inputs(x, h, ws_w, qa_q, qa_wk, qa_wv, mn_w, hn_w):
    """Host-side slicing/transposes -> per-core input maps."""
    bsz = x.shape[0]
    xp = np.zeros((bsz, CONV, DIM), np.float32)
    xp[:, :x.shape[1], :] = x
    wsT_tiles = ws_w.T.reshape(128, 128, 1024)       # f-tile index l*8+dc
    k = np.arange(128)
    perm = (k % 16) * 8 + (k // 16)                  # k-th processed tile -> f-tile
    wt = np.ascontiguousarray(
        wsT_tiles[perm].reshape(32, 4, 128, 1024).transpose(0, 2, 1, 3)
        .reshape(32, 128, 4096)).astype(_NPDT)
    wv = np.ascontiguousarray(qa_wv.T).reshape(8, 128, 1024).astype(_NPDT)
    cq = np.ascontiguousarray(
        ((qa_q.astype(np.float64) / np.sqrt(np.float64(DIM))).astype(np.float32)
         @ qa_wk).T).reshape(8, 128, 64).astype(_NPDT)
    ident = np.eye(64, dtype=np.float32)
    in_maps = []
    for c in range(N_CORES):
        b, half = c // 2, c % 2
        p0 = half * NLOC * STRIDE
        xt = np.ascontiguousarray(
            xp[b, p0:p0 + XLOC, :].T).reshape(8, 128, XLOC).astype(_NPDT)
        in_maps.append({
            "xt": xt, "wt": wt, "wv": wv, "cq": cq,
            "hb": np.ascontiguousarray(h[b]),
            "mnw": np.ascontiguousarray(mn_w),
            "hnw": np.ascontiguousarray(hn_w),
            "ident": ident,
        })
    return in_maps


_NC_CACHE = {}


def kernel(x, h, ws_w, sa_wq, sa_wk, sa_wv, qa_q, qa_wk, qa_wv, mn_w, hn_w):
    if "nc" not in _NC_CACHE:
        _NC_CACHE["nc"] = build_nc(reps=1, use_collective=True)
    nc = _NC_CACHE["nc"]
    in_maps = prep_inputs(x, h, ws_w, qa_q, qa_wk, qa_wv, mn_w, hn_w)
    res = bass_utils.run_bass_kernel_spmd(nc, in_maps, core_ids=list(range(N_CORES)))
    out = np.stack([res.results[2 * b]["out"] for b in range(4)], axis=0)
    return out.astype(np.float32)
